# revision 1
# baseline (speedup 1.0000x reference)
"""Trainium2 Bass kernel for the MAMCA model (conv frontend + 2 Mamba blocks + head).

Sharding: data-parallel over batch. 16 batches / 8 cores = 2 per core, no
collectives. Each core runs the full network on its batch shard; host
concatenates the per-core [2, 32] outputs.

On-chip layout is [channel -> partitions, time -> free dim] throughout.
The selective scan uses the DVE TensorTensorScan instruction (one linear
recurrence per partition along the free dim), time-chunked with state
carried across chunks via the scan's `initial` operand.
"""

import os
import numpy as np
import ml_dtypes

import concourse.bass as bass
import concourse.mybir as mybir
import concourse.tile as tile
from concourse import bacc
from concourse.bass_utils import run_bass_kernel_spmd

AF = mybir.ActivationFunctionType
ALU = mybir.AluOpType
F32 = mybir.dt.float32
BF16 = mybir.dt.bfloat16
NPBF = ml_dtypes.bfloat16

# Model dims (hardcoded; must match the grader's setup_inputs)
B, CIN, L = 16, 6, 2048
D, E, N, R, DCONV = 512, 1024, 16, 32, 4
NL, NCLS = 2, 32
RMS_EPS = 1e-5
BN_EPS = 1e-5

NCORES = 8
BC = B // NCORES          # batches per core
TC = 512                  # time chunk
NCH = L // TC
DT_T = D // 128           # 4 d_model partition tiles
ET = E // 128             # 8 d_inner partition tiles
MT = 2 * E // 128         # 16 in_proj output tiles
NG = 2                    # scan n-groups
GS = N // NG              # states per group


def _col(ap):
    """View a 1-D AP [P] as [P, 1]."""
    return ap.rearrange("(p one) -> p one", one=1)


def _declare(nc):
    dd = {}

    def din(name, shape, dt=F32):
        dd[name] = nc.declare_dram_parameter(name, list(shape), dt, isOutput=False)
        return dd[name]

    din("xs", (BC, CIN, L))
    din("dww", (CIN, 3))
    din("pwT", (CIN, D))
    din("bns", (D,))
    din("bnb", (D,))
    din("nw", (NL, D))
    din("inwT", (NL, D, 2 * E), BF16)
    din("cwdiag", (NL, E // 128, 128, DCONV * 128), BF16)
    din("cb", (NL, E))
    din("xpwT", (NL, E, 96), BF16)
    din("dtpwT", (NL, R, E), BF16)
    din("dtpb", (NL, E))
    din("A", (NL, E, N))
    din("Dp", (NL, E))
    din("owT", (NL, E, D), BF16)
    din("nfw", (D,))
    din("fcwT", (D, NCLS))
    din("fcb", (NCLS,))
    din("ones", (128, 128), BF16)
    din("sel", (N, N * 128), BF16)
    out = nc.declare_dram_parameter("out", [BC, NCLS], F32, isOutput=True)
    return dd, out


def _body(tc, dd, out_dram):
    from contextlib import ExitStack
    stack = ExitStack()
    nc = tc.nc
    dma = nc.sync.dma_start

    cst = stack.enter_context(tc.tile_pool(name="cst", bufs=1))
    wl = stack.enter_context(tc.tile_pool(name="wl", bufs=1))
    hres = stack.enter_context(tc.tile_pool(name="hres", bufs=1))
    ck = stack.enter_context(tc.tile_pool(name="ck", bufs=1))
    slabp = stack.enter_context(tc.tile_pool(name="slabp", bufs=2))
    bcp = stack.enter_context(tc.tile_pool(name="bcp", bufs=1))
    sc = stack.enter_context(tc.tile_pool(name="sc", bufs=2))
    hst = stack.enter_context(tc.tile_pool(name="hst", bufs=8))
    psA = stack.enter_context(
        tc.tile_pool(name="psA", bufs=6, space=bass.MemorySpace.PSUM))

    # ---- constants ----
    zero_c = cst.tile([128, 1], F32, name="zero_c")
    nc.gpsimd.memset(zero_c[:], 0.0)
    eps_c = cst.tile([128, 1], F32, name="eps_c")
    nc.gpsimd.memset(eps_c[:], RMS_EPS)
    one_c = cst.tile([128, 1], F32, name="one_c")
    nc.gpsimd.memset(one_c[:], 1.0)
    nc.const_aps.aps[(F32, 0.0)] = zero_c[:]
    nc.const_aps.aps[(F32, RMS_EPS)] = eps_c[:]
    nc.const_aps.aps[(F32, 1.0)] = one_c[:]
    ones_t = cst.tile([128, 128], BF16, name="ones_t")
    dma(ones_t[:], dd["ones"][:])
    sel_t = cst.tile([N, N * 128], BF16, name="sel_t")
    dma(sel_t[:], dd["sel"][:])
    pw_t = cst.tile([CIN, D], F32, name="pw_t")
    dma(pw_t[:], dd["pwT"][:])
    dww_t = cst.tile([CIN, 3], F32, name="dww_t")
    dma(dww_t[:], dd["dww"][:])
    bns_c, bnb_c, nfw_c, fcw_c = [], [], [], []
    for d in range(DT_T):
        sl = slice(d * 128, (d + 1) * 128)
        t1 = cst.tile([128, 1], F32, name=f"bns{d}")
        dma(t1[:], _col(dd["bns"][sl]))
        bns_c.append(t1)
        t2 = cst.tile([128, 1], F32, name=f"bnb{d}")
        dma(t2[:], _col(dd["bnb"][sl]))
        bnb_c.append(t2)
        t3 = cst.tile([128, 1], F32, name=f"nfw{d}")
        dma(t3[:], _col(dd["nfw"][sl]))
        nfw_c.append(t3)
        t4 = cst.tile([128, NCLS], F32, name=f"fcw{d}")
        dma(t4[:], dd["fcwT"][sl, :])
        fcw_c.append(t4)
    fcb_t = cst.tile([NCLS, 1], F32, name="fcb_t")
    dma(fcb_t[:], _col(dd["fcb"][:]))

    for b in range(BC):
        # ================= frontend (chunked over t, halo=1) =================
        h = []
        for d in range(DT_T):
            ht = hres.tile([128, L], BF16, tag=f"h{d}", name=f"h{d}")
            h.append(ht)
        for c in range(NCH):
            tsl = slice(c * TC, (c + 1) * TC)
            xpc = ck.tile([CIN, TC + 2], F32, tag="xpad", bufs=2, name="xpad")
            lo = c * TC - 1
            hi = (c + 1) * TC + 1
            if c == 0:
                nc.gpsimd.memset(xpc[:, 0:1], 0.0)
                dma(xpc[:, 1 : TC + 2], dd["xs"][b][:, 0 : TC + 1])
            elif c == NCH - 1:
                nc.gpsimd.memset(xpc[:, TC + 1 : TC + 2], 0.0)
                dma(xpc[:, 0 : TC + 1], dd["xs"][b][:, lo:L])
            else:
                dma(xpc[:], dd["xs"][b][:, lo:hi])
            dwc = ck.tile([CIN, TC], F32, tag="dwc", bufs=2, name="dwc")
            nc.vector.tensor_scalar(dwc[:], xpc[:, 0:TC], dww_t[:, 0:1],
                                    None, ALU.mult)
            for k in (1, 2):
                nc.vector.scalar_tensor_tensor(
                    dwc[:], xpc[:, k : k + TC], dww_t[:, k : k + 1], dwc[:],
                    ALU.mult, ALU.add)
            for d in range(DT_T):
                ps = psA.tile([128, TC], F32, tag="mm", name="psf")
                nc.tensor.matmul(ps[:], pw_t[:, d * 128 : (d + 1) * 128],
                                 dwc[:], start=True, stop=True)
                nc.scalar.activation(h[d][:, tsl], ps[:], AF.Gelu,
                                     bias=bnb_c[d][:, 0:1], scale=bns_c[d][:, 0:1])

        # ================= mamba layers =================
        for l in range(NL):
            # ---- per-layer weights ----
            inw_s = []
            for kt in range(DT_T):
                w = wl.tile([128, 2 * E], BF16, tag=f"inw{kt}", name=f"inw{kt}")
                dma(w[:], dd["inwT"][l, kt * 128 : (kt + 1) * 128, :])
                inw_s.append(w)
            ow_s, xpw_s, A_s, cw_s, cb_s, dtpb_s, D_s = [], [], [], [], [], [], []
            for et in range(ET):
                sl = slice(et * 128, (et + 1) * 128)
                w = wl.tile([128, D], BF16, tag=f"ow{et}", name=f"ow{et}")
                dma(w[:], dd["owT"][l, sl, :])
                ow_s.append(w)
                w = wl.tile([128, 96], BF16, tag=f"xpw{et}", name=f"xpw{et}")
                dma(w[:], dd["xpwT"][l, sl, :])
                xpw_s.append(w)
                w = wl.tile([128, N], F32, tag=f"A{et}", name=f"A{et}")
                dma(w[:], dd["A"][l, sl, :])
                A_s.append(w)
                w = wl.tile([128, DCONV * 128], BF16, tag=f"cw{et}", name=f"cw{et}")
                dma(w[:], dd["cwdiag"][l, et])
                cw_s.append(w)
                w = wl.tile([128, 1], F32, tag=f"cb{et}", name=f"cb{et}")
                dma(w[:], _col(dd["cb"][l, sl]))
                cb_s.append(w)
                w = wl.tile([128, 1], F32, tag=f"dtpb{et}", name=f"dtpb{et}")
                dma(w[:], _col(dd["dtpb"][l, sl]))
                dtpb_s.append(w)
                w = wl.tile([128, 1], F32, tag=f"D{et}", name=f"D{et}")
                dma(w[:], _col(dd["Dp"][l, sl]))
                D_s.append(w)
            dtpw_s = wl.tile([R, E], BF16, tag="dtpw", name="dtpw")
            dma(dtpw_s[:], dd["dtpwT"][l])
            nw_c = []
            for d in range(DT_T):
                w = wl.tile([128, 1], F32, tag=f"nw{d}", name=f"nw{d}")
                dma(w[:], _col(dd["nw"][l, d * 128 : (d + 1) * 128]))
                nw_c.append(w)

            # ---- per-(b,l) state ----
            hstate, ucarry = [], []
            for et in range(ET):
                s = hst.tile([128, N], F32, tag=f"hs{et}", name=f"hs{et}")
                hstate.append(s)
                uca = hst.tile([128, 3], BF16, tag=f"uca{et}", name=f"uca{et}")
                nc.gpsimd.memset(uca[:], 0.0)
                ucarry.append(uca)

            for c in range(NCH):
                tsl = slice(c * TC, (c + 1) * TC)
                # ---- rmsnorm ----
                ssp = psA.tile([128, TC], F32, tag="mm", name="ssp")
                for d in range(DT_T):
                    sq = sc.tile([128, TC], BF16, tag="sq", name="sq")
                    nc.scalar.activation(sq[:], h[d][:, tsl], AF.Square)
                    nc.tensor.matmul(ssp[:], ones_t[:], sq[:],
                                     start=(d == 0), stop=(d == DT_T - 1))
                invb = sc.tile([128, TC], F32, tag="invb", name="invb")
                nc.scalar.activation(invb[:], ssp[:], AF.Abs_reciprocal_sqrt,
                                     bias=RMS_EPS, scale=1.0 / D)
                hn = []
                for d in range(DT_T):
                    t = sc.tile([128, TC], BF16, tag=f"hn{d}", name=f"hn{d}")
                    nc.vector.scalar_tensor_tensor(
                        t[:], h[d][:, tsl], nw_c[d][:, 0:1], invb[:],
                        ALU.mult, ALU.mult)
                    hn.append(t)

                # ---- in_proj -> u (conv+silu) and z (silu) ----
                uc, sz = [None] * ET, [None] * ET
                for mt in range(MT):
                    ps = psA.tile([128, TC], F32, tag="mm", name="psip")
                    for kt in range(DT_T):
                        nc.tensor.matmul(
                            ps[:], inw_s[kt][:, mt * 128 : (mt + 1) * 128],
                            hn[kt][:], start=(kt == 0), stop=(kt == DT_T - 1))
                    if mt < ET:
                        et = mt
                        upad = ck.tile([128, TC + 3], BF16, tag="upad", bufs=3,
                                       name="upad")
                        nc.gpsimd.tensor_copy(upad[:, 0:3], ucarry[et][:])
                        nc.scalar.activation(upad[:, 3 : TC + 3], ps[:], AF.Copy)
                        if c < NCH - 1:
                            nc.gpsimd.tensor_copy(ucarry[et][:], upad[:, TC : TC + 3])
                        psc = psA.tile([128, TC], F32, tag="mm", name="psc")
                        for k in range(DCONV):
                            nc.tensor.matmul(
                                psc[:], cw_s[et][:, k * 128 : (k + 1) * 128],
                                upad[:, k : k + TC], start=(k == 0),
                                stop=(k == DCONV - 1))
                        t = ck.tile([128, TC], BF16, tag=f"uc{et}", name=f"uc{et}")
                        nc.scalar.activation(t[:], psc[:], AF.Silu,
                                             bias=cb_s[et][:, 0:1])
                        uc[et] = t
                    else:
                        et = mt - ET
                        t = ck.tile([128, TC], BF16, tag=f"sz{et}", name=f"sz{et}")
                        nc.scalar.activation(t[:], ps[:], AF.Silu)
                        sz[et] = t

                # ---- x_proj ----
                psx = psA.tile([96, TC], F32, tag="mm", name="psx")
                for kt in range(ET):
                    nc.tensor.matmul(psx[:], xpw_s[kt][:], uc[kt][:],
                                     start=(kt == 0), stop=(kt == ET - 1))
                dtraw = sc.tile([R, TC], BF16, tag="dtraw", name="dtraw")
                nc.scalar.activation(dtraw[:], psx[0:R, :], AF.Copy)
                Bt = sc.tile([N, TC], BF16, tag="Bt", name="Bt")
                nc.scalar.activation(Bt[:], psx[R : R + N, :], AF.Copy)
                Ct = sc.tile([N, TC], BF16, tag="Ct", name="Ct")
                nc.scalar.activation(Ct[:], psx[64:80, :], AF.Copy)

                # ---- dt_proj + softplus; dtu ----
                dtt, dtu = [None] * ET, [None] * ET
                for et in range(ET):
                    ps = psA.tile([128, TC], F32, tag="mm", name="psdt")
                    nc.tensor.matmul(
                        ps[:], dtpw_s[:, et * 128 : (et + 1) * 128], dtraw[:],
                        start=True, stop=True)
                    edt = sc.tile([128, TC], F32, tag="edt", name="edt")
                    nc.scalar.activation(edt[:], ps[:], AF.Exp,
                                         bias=dtpb_s[et][:, 0:1])
                    t = ck.tile([128, TC], BF16, tag=f"dt{et}", name=f"dt{et}")
                    nc.scalar.activation(t[:], edt[:], AF.Ln, bias=1.0)
                    dtt[et] = t
                    t2 = ck.tile([128, TC], BF16, tag=f"dtu{et}", name=f"dtu{et}")
                    nc.vector.tensor_mul(t2[:], t[:], uc[et][:])
                    dtu[et] = t2

                # ---- broadcast B, C across partitions ----
                Bb = bcp.tile([128, N * TC], BF16, tag="Bb", name="Bb")
                Cb = bcp.tile([128, N * TC], BF16, tag="Cb", name="Cb")
                for n in range(N):
                    nsl = slice(n * TC, (n + 1) * TC)
                    lsl = slice(n * 128, (n + 1) * 128)
                    ps = psA.tile([128, TC], F32, tag="mm", name="psbb")
                    nc.tensor.matmul(ps[:], sel_t[:, lsl], Bt[:],
                                     start=True, stop=True)
                    nc.scalar.activation(Bb[:, nsl], ps[:], AF.Copy)
                    ps2 = psA.tile([128, TC], F32, tag="mm", name="pscb")
                    nc.tensor.matmul(ps2[:], sel_t[:, lsl], Ct[:],
                                     start=True, stop=True)
                    nc.scalar.activation(Cb[:, nsl], ps2[:], AF.Copy)

                # ---- selective scan per e-tile, n in NG groups ----
                yg = [None] * ET
                for et in range(ET):
                    y = sc.tile([128, TC], BF16, tag="y", name="y")
                    for g in range(NG):
                        slab = slabp.tile([128, GS * TC], BF16, tag="slab",
                                          name="slab")
                        for j in range(GS):
                            n = g * GS + j
                            jsl = slice(j * TC, (j + 1) * TC)
                            nsl = slice(n * TC, (n + 1) * TC)
                            dA = sc.tile([128, TC], BF16, tag="dA", name="dA")
                            nc.scalar.activation(dA[:], dtt[et][:], AF.Exp,
                                                 scale=A_s[et][:, n : n + 1])
                            X = sc.tile([128, TC], BF16, tag="X", name="X")
                            nc.vector.tensor_mul(X[:], dtu[et][:], Bb[:, nsl])
                            init = 0.0 if c == 0 else hstate[et][:, n : n + 1]
                            nc.vector.tensor_tensor_scan(
                                slab[:, jsl], dA[:], X[:], init, ALU.mult, ALU.add)
                        if c < NCH - 1:
                            nc.gpsimd.tensor_copy(
                                hstate[et][:, g * GS : (g + 1) * GS].rearrange(
                                    "p (n one) -> p n one", one=1),
                                slab[:].rearrange("p (n t) -> p n t", n=GS)
                                [:, :, TC - 1 : TC])
                        nc.vector.tensor_mul(
                            slab[:], slab[:], Cb[:, g * GS * TC : (g + 1) * GS * TC])
                        if g == 0:
                            nc.gpsimd.tensor_add(y[:], slab[:, 0:TC],
                                                 slab[:, TC : 2 * TC])
                            rng = range(2, GS)
                        else:
                            rng = range(GS)
                        for j in rng:
                            nc.gpsimd.tensor_add(
                                y[:], y[:], slab[:, j * TC : (j + 1) * TC])
                    t = sc.tile([128, TC], BF16, tag=f"yg{et}", bufs=1,
                                name=f"yg{et}")
                    nc.vector.scalar_tensor_tensor(
                        t[:], uc[et][:], D_s[et][:, 0:1], y[:], ALU.mult, ALU.add)
                    nc.vector.tensor_mul(t[:], t[:], sz[et][:])
                    yg[et] = t

                # ---- out_proj + residual ----
                for d in range(DT_T):
                    ps = psA.tile([128, TC], F32, tag="mm", name="psop")
                    for kt in range(ET):
                        nc.tensor.matmul(
                            ps[:], ow_s[kt][:, d * 128 : (d + 1) * 128], yg[kt][:],
                            start=(kt == 0), stop=(kt == ET - 1))
                    nc.vector.tensor_add(h[d][:, tsl], h[d][:, tsl], ps[:])

        # ================= head =================
        hmacc = []
        for d in range(DT_T):
            t = sc.tile([128, NCH], F32, tag=f"hm{d}", name=f"hm{d}")
            hmacc.append(t)
        for c in range(NCH):
            tsl = slice(c * TC, (c + 1) * TC)
            ssp = psA.tile([128, TC], F32, tag="mm", name="ssf")
            for d in range(DT_T):
                sq = sc.tile([128, TC], BF16, tag="sq", name="sqf")
                nc.scalar.activation(sq[:], h[d][:, tsl], AF.Square)
                nc.tensor.matmul(ssp[:], ones_t[:], sq[:],
                                 start=(d == 0), stop=(d == DT_T - 1))
            invb = sc.tile([128, TC], F32, tag="invb", name="invbf")
            nc.scalar.activation(invb[:], ssp[:], AF.Abs_reciprocal_sqrt,
                                 bias=RMS_EPS, scale=1.0 / D)
            for d in range(DT_T):
                hnf = sc.tile([128, TC], F32, tag="hnf", name="hnf")
                nc.vector.scalar_tensor_tensor(
                    hnf[:], h[d][:, tsl], nfw_c[d][:, 0:1], invb[:],
                    ALU.mult, ALU.mult)
                nc.vector.tensor_reduce(hmacc[d][:, c : c + 1], hnf[:],
                                        mybir.AxisListType.X, ALU.add)
        psf = psA.tile([NCLS, 1], F32, tag="mm", name="psfc")
        for d in range(DT_T):
            hms = sc.tile([128, 1], F32, tag="hms", name="hms")
            nc.vector.tensor_reduce(hms[:], hmacc[d][:], mybir.AxisListType.X, ALU.add)
            nc.tensor.matmul(psf[:], fcw_c[d][:], hms[:],
                             start=(d == 0), stop=(d == DT_T - 1))
        ot = sc.tile([NCLS, 1], F32, tag="ot", name="ot")
        nc.vector.scalar_tensor_tensor(ot[:], psf[:], 1.0 / L, fcb_t[:],
                                       ALU.mult, ALU.add)
        dma(out_dram[b].rearrange("(a one) -> a one", one=1), ot[:])

    stack.close()




def _conv_diag(cw):
    # conv_w [NL, E, 1, DCONV] -> per (l, e-tile, k) a 128x128 diagonal block:
    # out[l, et, p, k*128 + q] = cw[l, et*128+p, 0, k] if p == q else 0
    et = E // 128
    out = np.zeros((NL, et, 128, DCONV * 128), np.float32)
    for l in range(NL):
        for t in range(et):
            for k in range(DCONV):
                blk = np.diag(cw[l, t * 128 : (t + 1) * 128, 0, k])
                out[l, t, :, k * 128 : (k + 1) * 128] = blk
    return np.ascontiguousarray(out).astype(NPBF)


def _pad_xpw(xp):
    # [NL, 64, E] -> transposed+padded [NL, E, 96]: cols 0:48 = dt_raw+B,
    # cols 64:80 = C (PSUM partition bases must be multiples of 32)
    t = np.transpose(xp, (0, 2, 1))
    out = np.zeros((t.shape[0], t.shape[1], 96), t.dtype)
    out[:, :, 0:48] = t[:, :, 0:48]
    out[:, :, 64:80] = t[:, :, 48:64]
    return out


def _prep_inputs(inp):
    f32 = np.float32
    ca = np.ascontiguousarray

    def bf(a):
        return ca(np.asarray(a, f32)).astype(NPBF)

    bns = np.asarray(inp["bn_w"], f32) / np.sqrt(np.asarray(inp["bn_var"], f32) + BN_EPS)
    bnb = np.asarray(inp["bn_b"], f32) - np.asarray(inp["bn_mean"], f32) * bns
    base = {
        "dww": ca(np.asarray(inp["dw_w"], f32)[:, 0, :]),
        "pwT": ca(np.asarray(inp["pw_w"], f32)[:, :, 0].T),
        "bns": ca(bns),
        "bnb": ca(bnb),
        "nw": ca(np.asarray(inp["norm_w"], f32)),
        "inwT": bf(np.transpose(np.asarray(inp["in_proj_w"], f32), (0, 2, 1))),
        "cwdiag": _conv_diag(np.asarray(inp["conv_w"], f32)),
        "cb": ca(np.asarray(inp["conv_b"], f32)),
        "xpwT": bf(_pad_xpw(np.asarray(inp["xproj_w"], f32))),
        "dtpwT": bf(np.transpose(np.asarray(inp["dtproj_w"], f32), (0, 2, 1))),
        "dtpb": ca(np.asarray(inp["dtproj_b"], f32)),
        "A": ca(-np.exp(np.asarray(inp["A_log"], f32))),
        "Dp": ca(np.asarray(inp["D"], f32)),
        "owT": bf(np.transpose(np.asarray(inp["outproj_w"], f32), (0, 2, 1))),
        "nfw": ca(np.asarray(inp["normf_w"], f32)),
        "fcwT": ca(np.asarray(inp["fc_w"], f32).T),
        "fcb": ca(np.asarray(inp["fc_b"], f32)),
        "ones": np.ones((128, 128), NPBF),
        "sel": ca(np.repeat(np.eye(N, dtype=f32), 128, axis=1)).astype(NPBF),
    }
    x = np.asarray(inp["x"], f32)
    in_maps = []
    for i in range(NCORES):
        m = dict(base)
        m["xs"] = ca(x[i * BC : (i + 1) * BC])
        in_maps.append(m)
    return in_maps


_LAST_RESULTS = {}


def kernel(**inputs):
    nc = bacc.Bacc("TRN2", target_bir_lowering=False, debug=False,
                   num_devices=NCORES)
    dd, out_d = _declare(nc)
    with tile.TileContext(nc) as tcx:
        _body(tcx, dd, out_d)
    nc.compile()

    in_maps = _prep_inputs(inputs)
    trace = bool(os.environ.get("BASS_KERNEL_TRACE"))
    res = run_bass_kernel_spmd(nc, in_maps, list(range(NCORES)), trace=trace)
    _LAST_RESULTS["res"] = res
    if res.exec_time_ns is not None:
        print(f"HW exec time: {res.exec_time_ns} ns")
    out = np.concatenate(
        [np.asarray(res.results[i]["out"], np.float32) for i in range(NCORES)], axis=0)
    return out



# revision 8
# speedup vs baseline: 1.4079x; 1.4079x over previous
"""Trainium2 Bass kernel for the MAMCA model (conv frontend + 2 Mamba blocks + head).

Sharding: data-parallel over batch. 16 batches / 8 cores = 2 per core, no
collectives. Each core runs the full network on its batch shard; host
concatenates the per-core [2, 32] outputs.

On-chip layout is [channel -> partitions, time -> free dim] throughout.

Selective scan: per (et, chunk) the 16 states are processed as ONE
tensor_tensor_scan over [128, 16*512] with per-segment isolation (dA zeroed
at segment starts, chunk carry injected into X[segment_start]).  The
per-state y contraction sum_n C_n * h_n runs on the PE as 16 chained
identity matmuls accumulating into one PSUM bank.  Elementwise X/C products
are single big DVE tensor_tensor ops (broadcast APs avoid materializing
replicas).  Copies / small strided fixups live on GpSimd; ACT instructions
are emitted grouped by activation function to avoid table reloads.
"""

import os
import numpy as np
import ml_dtypes

import concourse.bass as bass
import concourse.mybir as mybir
import concourse.tile as tile
from concourse import bacc
from concourse.bass_utils import run_bass_kernel_spmd

AF = mybir.ActivationFunctionType
ALU = mybir.AluOpType
F32 = mybir.dt.float32
BF16 = mybir.dt.bfloat16
NPBF = ml_dtypes.bfloat16

# Model dims (hardcoded; must match the grader's setup_inputs)
B, CIN, L = 16, 6, 2048
D, E, N, R, DCONV = 512, 1024, 16, 32, 4
NL, NCLS = 2, 32
RMS_EPS = 1e-5
BN_EPS = 1e-5

NCORES = 8
BC = B // NCORES          # batches per core
TC = 512                  # time chunk
NCH = L // TC
DT_T = D // 128           # 4 d_model partition tiles
ET = E // 128             # 8 d_inner partition tiles
MT = 2 * E // 128         # 16 in_proj output tiles
NT = N * TC               # merged scan free size


def _col(ap):
    """View a 1-D AP [P] as [P, 1]."""
    return ap.rearrange("(p one) -> p one", one=1)


def _declare(nc):
    dd = {}

    def din(name, shape, dt=F32):
        dd[name] = nc.declare_dram_parameter(name, list(shape), dt, isOutput=False)
        return dd[name]

    din("xs", (BC, CIN, L))
    din("dww", (CIN, 3))
    din("pwT", (CIN, D))
    din("bns", (D,))
    din("bnb", (D,))
    din("nw", (NL, D))
    din("inwT", (NL, D, 2 * E), BF16)
    din("cwdiag", (NL, E // 128, 128, DCONV * 128), BF16)
    din("cb", (NL, E))
    din("xpwT", (NL, E, 96), BF16)
    din("dtpwT", (NL, R, E), BF16)
    din("dtpb", (NL, E))
    din("A", (NL, E, N))
    din("Dp", (NL, E))
    din("owT", (NL, E, D), BF16)
    din("nfw", (D,))
    din("fcwT", (D, NCLS))
    din("fcb", (NCLS,))
    din("ones", (128, 128), BF16)
    din("ident", (128, 128), BF16)
    din("sel", (N, N * 128), BF16)
    out = nc.declare_dram_parameter("out", [BC, NCLS], F32, isOutput=True)
    return dd, out


def _body(tc, dd, out_dram):
    from contextlib import ExitStack
    stack = ExitStack()
    nc = tc.nc
    dma = nc.sync.dma_start

    cst = stack.enter_context(tc.tile_pool(name="cst", bufs=1))
    wl = stack.enter_context(tc.tile_pool(name="wl", bufs=1))
    hres = stack.enter_context(tc.tile_pool(name="hres", bufs=1))
    ck = stack.enter_context(tc.tile_pool(name="ck", bufs=1))
    big = stack.enter_context(tc.tile_pool(name="big", bufs=2))
    bcp = stack.enter_context(tc.tile_pool(name="bcp", bufs=1))
    sc = stack.enter_context(tc.tile_pool(name="sc", bufs=2))
    hst = stack.enter_context(tc.tile_pool(name="hst", bufs=1))
    psA = stack.enter_context(
        tc.tile_pool(name="psA", bufs=3, space=bass.MemorySpace.PSUM))
    psY = stack.enter_context(
        tc.tile_pool(name="psY", bufs=2, space=bass.MemorySpace.PSUM))
    psO = stack.enter_context(
        tc.tile_pool(name="psO", bufs=3, space=bass.MemorySpace.PSUM))

    # ---- constants ----
    zero_c = cst.tile([128, 1], F32, name="zero_c")
    nc.gpsimd.memset(zero_c[:], 0.0)
    eps_c = cst.tile([128, 1], F32, name="eps_c")
    nc.gpsimd.memset(eps_c[:], RMS_EPS)
    one_c = cst.tile([128, 1], F32, name="one_c")
    nc.gpsimd.memset(one_c[:], 1.0)
    nc.const_aps.aps[(F32, 0.0)] = zero_c[:]
    nc.const_aps.aps[(F32, RMS_EPS)] = eps_c[:]
    nc.const_aps.aps[(F32, 1.0)] = one_c[:]
    ones_t = cst.tile([128, 128], BF16, name="ones_t")
    dma(ones_t[:], dd["ones"][:])
    ident_t = cst.tile([128, 128], BF16, name="ident_t")
    dma(ident_t[:], dd["ident"][:])
    sel_t = cst.tile([N, N * 128], BF16, name="sel_t")
    dma(sel_t[:], dd["sel"][:])
    pw_t = cst.tile([CIN, D], F32, name="pw_t")
    dma(pw_t[:], dd["pwT"][:])
    dww_t = cst.tile([CIN, 3], F32, name="dww_t")
    dma(dww_t[:], dd["dww"][:])
    bns_c, bnb_c, nfw_c, fcw_c = [], [], [], []
    for d in range(DT_T):
        sl = slice(d * 128, (d + 1) * 128)
        t1 = cst.tile([128, 1], F32, name=f"bns{d}")
        dma(t1[:], _col(dd["bns"][sl]))
        bns_c.append(t1)
        t2 = cst.tile([128, 1], F32, name=f"bnb{d}")
        dma(t2[:], _col(dd["bnb"][sl]))
        bnb_c.append(t2)
        t3 = cst.tile([128, 1], F32, name=f"nfw{d}")
        dma(t3[:], _col(dd["nfw"][sl]))
        nfw_c.append(t3)
        t4 = cst.tile([128, NCLS], F32, name=f"fcw{d}")
        dma(t4[:], dd["fcwT"][sl, :])
        fcw_c.append(t4)
    fcb_t = cst.tile([NCLS, 1], F32, name="fcb_t")
    dma(fcb_t[:], _col(dd["fcb"][:]))

    for b in range(BC):
        # ================= frontend (chunked over t, halo=1) =================
        h = []
        for d in range(DT_T):
            ht = hres.tile([128, L], BF16, tag=f"h{d}", name=f"h{d}")
            h.append(ht)
        for c in range(NCH):
            tsl = slice(c * TC, (c + 1) * TC)
            xpc = ck.tile([CIN, TC + 2], F32, tag="xpad", bufs=1, name="xpad")
            lo = c * TC - 1
            hi = (c + 1) * TC + 1
            if c == 0:
                nc.gpsimd.memset(xpc[:, 0:1], 0.0)
                dma(xpc[:, 1 : TC + 2], dd["xs"][b][:, 0 : TC + 1])
            elif c == NCH - 1:
                nc.gpsimd.memset(xpc[:, TC + 1 : TC + 2], 0.0)
                dma(xpc[:, 0 : TC + 1], dd["xs"][b][:, lo:L])
            else:
                dma(xpc[:], dd["xs"][b][:, lo:hi])
            dwc = ck.tile([CIN, TC], F32, tag="dwc", bufs=1, name="dwc")
            nc.vector.tensor_scalar(dwc[:], xpc[:, 0:TC], dww_t[:, 0:1],
                                    None, ALU.mult)
            for k in (1, 2):
                nc.vector.scalar_tensor_tensor(
                    dwc[:], xpc[:, k : k + TC], dww_t[:, k : k + 1], dwc[:],
                    ALU.mult, ALU.add)
            for d in range(DT_T):
                ps = psA.tile([128, TC], F32, tag="mm", name="psf")
                nc.tensor.matmul(ps[:], pw_t[:, d * 128 : (d + 1) * 128],
                                 dwc[:], start=True, stop=True)
                nc.scalar.activation(h[d][:, tsl], ps[:], AF.Gelu,
                                     bias=bnb_c[d][:, 0:1], scale=bns_c[d][:, 0:1])

        # ================= mamba layers =================
        for l in range(NL):
            # ---- per-layer weights ----
            inw_s = []
            for kt in range(DT_T):
                w = wl.tile([128, 2 * E], BF16, tag=f"inw{kt}", name=f"inw{kt}")
                dma(w[:], dd["inwT"][l, kt * 128 : (kt + 1) * 128, :])
                inw_s.append(w)
            ow_s, xpw_s, A_s, cw_s, cb_s, dtpb_s, D_s = [], [], [], [], [], [], []
            for et in range(ET):
                sl = slice(et * 128, (et + 1) * 128)
                w = wl.tile([128, D], BF16, tag=f"ow{et}", name=f"ow{et}")
                dma(w[:], dd["owT"][l, sl, :])
                ow_s.append(w)
                w = wl.tile([128, 96], BF16, tag=f"xpw{et}", name=f"xpw{et}")
                dma(w[:], dd["xpwT"][l, sl, :])
                xpw_s.append(w)
                w = wl.tile([128, N], F32, tag=f"A{et}", name=f"A{et}")
                dma(w[:], dd["A"][l, sl, :])
                A_s.append(w)
                w = wl.tile([128, DCONV * 128], BF16, tag=f"cw{et}", name=f"cw{et}")
                dma(w[:], dd["cwdiag"][l, et])
                cw_s.append(w)
                w = wl.tile([128, 1], F32, tag=f"cb{et}", name=f"cb{et}")
                dma(w[:], _col(dd["cb"][l, sl]))
                cb_s.append(w)
                w = wl.tile([128, 1], F32, tag=f"dtpb{et}", name=f"dtpb{et}")
                dma(w[:], _col(dd["dtpb"][l, sl]))
                dtpb_s.append(w)
                w = wl.tile([128, 1], F32, tag=f"D{et}", name=f"D{et}")
                dma(w[:], _col(dd["Dp"][l, sl]))
                D_s.append(w)
            dtpw_s = wl.tile([R, E], BF16, tag="dtpw", name="dtpw")
            dma(dtpw_s[:], dd["dtpwT"][l])
            nw_c = []
            for d in range(DT_T):
                w = wl.tile([128, 1], F32, tag=f"nw{d}", name=f"nw{d}")
                dma(w[:], _col(dd["nw"][l, d * 128 : (d + 1) * 128]))
                nw_c.append(w)

            # ---- per-(b,l) state ----
            hstate = []
            for et in range(ET):
                s = hst.tile([128, N], BF16, tag=f"hs{et}", name=f"hs{et}")
                hstate.append(s)
            ucarry = []
            for et in range(ET):
                uca = hst.tile([128, 3], BF16, tag=f"uca{et}", name=f"uca{et}")
                nc.gpsimd.memset(uca[:], 0.0)
                ucarry.append(uca)

            for c in range(NCH):
                tsl = slice(c * TC, (c + 1) * TC)
                # ---- rmsnorm ----
                ssp = psA.tile([128, TC], F32, tag="mm", name="ssp")
                for d in range(DT_T):
                    sq = sc.tile([128, TC], BF16, tag="sq", bufs=1, name="sq")
                    nc.scalar.activation(sq[:], h[d][:, tsl], AF.Square)
                    nc.tensor.matmul(ssp[:], ones_t[:], sq[:],
                                     start=(d == 0), stop=(d == DT_T - 1))
                invb = sc.tile([128, TC], F32, tag="invb", bufs=1, name="invb")
                nc.scalar.activation(invb[:], ssp[:], AF.Abs_reciprocal_sqrt,
                                     bias=RMS_EPS, scale=1.0 / D)
                hn = []
                for d in range(DT_T):
                    t = sc.tile([128, TC], BF16, tag=f"hn{d}", bufs=1, name=f"hn{d}")
                    nc.vector.scalar_tensor_tensor(
                        t[:], h[d][:, tsl], nw_c[d][:, 0:1], invb[:],
                        ALU.mult, ALU.mult)
                    hn.append(t)

                # ---- in_proj -> u (conv+silu) and z (silu) ----
                uc, sz = [None] * ET, [None] * ET
                for mt in range(MT):
                    ps = psA.tile([128, TC], F32, tag="mm", name="psip")
                    for kt in range(DT_T):
                        nc.tensor.matmul(
                            ps[:], inw_s[kt][:, mt * 128 : (mt + 1) * 128],
                            hn[kt][:], start=(kt == 0), stop=(kt == DT_T - 1))
                    if mt < ET:
                        et = mt
                        upad = ck.tile([128, TC + 3], BF16, tag="upad", bufs=2,
                                       name="upad")
                        nc.gpsimd.tensor_copy(upad[:, 0:3], ucarry[et][:])
                        nc.scalar.activation(upad[:, 3 : TC + 3], ps[:], AF.Copy)
                        if c < NCH - 1:
                            nc.gpsimd.tensor_copy(ucarry[et][:], upad[:, TC : TC + 3])
                        psc = psA.tile([128, TC], F32, tag="mm", name="psc")
                        for k in range(DCONV):
                            nc.tensor.matmul(
                                psc[:], cw_s[et][:, k * 128 : (k + 1) * 128],
                                upad[:, k : k + TC], start=(k == 0),
                                stop=(k == DCONV - 1))
                        t = ck.tile([128, TC], BF16, tag=f"uc{et}", name=f"uc{et}")
                        nc.scalar.activation(t[:], psc[:], AF.Silu,
                                             bias=cb_s[et][:, 0:1])
                        uc[et] = t
                    else:
                        et = mt - ET
                        t = ck.tile([128, TC], BF16, tag=f"sz{et}", name=f"sz{et}")
                        nc.scalar.activation(t[:], ps[:], AF.Silu)
                        sz[et] = t

                # ---- x_proj ----
                psx = psA.tile([96, TC], F32, tag="mm", name="psx")
                for kt in range(ET):
                    nc.tensor.matmul(psx[:], xpw_s[kt][:], uc[kt][:],
                                     start=(kt == 0), stop=(kt == ET - 1))
                dtraw = sc.tile([R, TC], BF16, tag="dtraw", bufs=1, name="dtraw")
                nc.scalar.activation(dtraw[:], psx[0:R, :], AF.Copy)
                Bt = sc.tile([N, TC], BF16, tag="Bt", bufs=1, name="Bt")
                nc.scalar.activation(Bt[:], psx[R : R + N, :], AF.Copy)
                Ct = sc.tile([N, TC], BF16, tag="Ct", bufs=1, name="Ct")
                nc.scalar.activation(Ct[:], psx[64:80, :], AF.Copy)

                # ---- dt_proj + softplus (Exp batch, then in-place Ln batch) ----
                dtt = [None] * ET
                for et in range(ET):
                    ps = psA.tile([128, TC], F32, tag="mm", name="psdt")
                    nc.tensor.matmul(
                        ps[:], dtpw_s[:, et * 128 : (et + 1) * 128], dtraw[:],
                        start=True, stop=True)
                    t = ck.tile([128, TC], BF16, tag=f"dt{et}", name=f"dt{et}")
                    nc.scalar.activation(t[:], ps[:], AF.Exp,
                                         bias=dtpb_s[et][:, 0:1])
                    dtt[et] = t
                for et in range(ET):
                    nc.scalar.activation(dtt[et][:], dtt[et][:], AF.Ln, bias=1.0)
                # ---- broadcast B, C across partitions (PE) ----
                Bb = bcp.tile([128, NT], BF16, tag="Bb", name="Bb")
                Cb = bcp.tile([128, NT], BF16, tag="Cb", name="Cb")
                for n in range(N):
                    nsl = slice(n * TC, (n + 1) * TC)
                    lsl = slice(n * 128, (n + 1) * 128)
                    ps = psA.tile([128, TC], F32, tag="mm", name="psbb")
                    nc.tensor.matmul(ps[:], sel_t[:, lsl], Bt[:],
                                     start=True, stop=True)
                    nc.scalar.activation(Bb[:, nsl], ps[:], AF.Copy)
                    ps2 = psA.tile([128, TC], F32, tag="mm", name="pscb")
                    nc.tensor.matmul(ps2[:], sel_t[:, lsl], Ct[:],
                                     start=True, stop=True)
                    nc.scalar.activation(Cb[:, nsl], ps2[:], AF.Copy)

                # ---- selective scan, merged over all 16 states per et ----
                yg = [None] * ET
                for et in range(ET):
                    # dA segments: exp(A_n * dt) for each state n
                    dA = big.tile([128, NT], BF16, tag="dA", name="dA")
                    for n in range(N):
                        nc.scalar.activation(dA[:, n * TC : (n + 1) * TC],
                                             dtt[et][:], AF.Exp,
                                             scale=A_s[et][:, n : n + 1])
                    dA3 = dA[:].rearrange("p (n t) -> p n t", n=N)
                    # dtt[et] -> dtu[et] in place (after the dA EXPs read it)
                    nc.vector.tensor_mul(dtt[et][:], dtt[et][:], uc[et][:])
                    if c > 0:
                        dA0 = sc.tile([128, N], F32, tag="dA0", name="dA0")
                        nc.gpsimd.tensor_copy(
                            dA0[:].rearrange("p (n one) -> p n one", one=1),
                            dA3[:, :, 0:1])
                    # zero segment starts so the scan restarts per state
                    nc.gpsimd.memset(dA3[:, :, 0:1], 0.0)

                    # X = dtu (broadcast over n) * Bb
                    xs = big.tile([128, NT], BF16, tag="xs", name="xs")
                    dtu_b = dtt[et][:].rearrange(
                        "p (one t) -> p one t", one=1).to_broadcast([128, N, TC])
                    nc.vector.tensor_tensor(
                        xs[:].rearrange("p (n t) -> p n t", n=N),
                        dtu_b, Bb[:].rearrange("p (n t) -> p n t", n=N),
                        ALU.mult)
                    xs3 = xs[:].rearrange("p (n t) -> p n t", n=N)
                    if c > 0:
                        # inject carry: X[:, n, 0] += dA0_n * hstate_n
                        tmp = sc.tile([128, N], F32, tag="ctmp", name="ctmp")
                        nc.gpsimd.tensor_mul(tmp[:], dA0[:], hstate[et][:])
                        nc.gpsimd.tensor_add(
                            xs3[:, :, 0:1],
                            xs3[:, :, 0:1],
                            tmp[:].rearrange("p (n one) -> p n one", one=1))

                    # the merged scan (state fp32 internally; dst in-place)
                    nc.vector.tensor_tensor_scan(
                        xs[:], dA[:], xs[:], 0.0, ALU.mult, ALU.add)

                    if c < NCH - 1:
                        nc.gpsimd.tensor_copy(
                            hstate[et][:].rearrange("p (n one) -> p n one", one=1),
                            xs3[:, :, TC - 1 : TC])

                    # multiply by C (in place) and reduce over n on the PE
                    nc.vector.tensor_mul(xs[:], xs[:], Cb[:])
                    psy = psY.tile([128, TC], F32, tag="y", name="psy")
                    for n in range(N):
                        nc.tensor.matmul(psy[:], ident_t[:],
                                         xs[:, n * TC : (n + 1) * TC],
                                         start=(n == 0), stop=(n == N - 1))

                    t = sc.tile([128, TC], BF16, tag=f"yg{et}", bufs=1,
                                name=f"yg{et}")
                    nc.vector.scalar_tensor_tensor(
                        t[:], uc[et][:], D_s[et][:, 0:1], psy[:],
                        ALU.mult, ALU.add)
                    nc.vector.tensor_mul(t[:], t[:], sz[et][:])
                    yg[et] = t

                # ---- out_proj + residual ----
                for d in range(DT_T):
                    ps = psO.tile([128, TC], F32, tag="op", name="psop")
                    for kt in range(ET):
                        nc.tensor.matmul(
                            ps[:], ow_s[kt][:, d * 128 : (d + 1) * 128],
                            yg[kt][:], start=(kt == 0), stop=(kt == ET - 1))
                    nc.vector.tensor_add(h[d][:, tsl], h[d][:, tsl], ps[:])

        # ================= head =================
        hmacc = []
        for d in range(DT_T):
            t = sc.tile([128, NCH], F32, tag=f"hm{d}", name=f"hm{d}")
            hmacc.append(t)
        for c in range(NCH):
            tsl = slice(c * TC, (c + 1) * TC)
            ssp = psA.tile([128, TC], F32, tag="mm", name="ssf")
            for d in range(DT_T):
                sq = sc.tile([128, TC], BF16, tag="sq", bufs=1, name="sqf")
                nc.scalar.activation(sq[:], h[d][:, tsl], AF.Square)
                nc.tensor.matmul(ssp[:], ones_t[:], sq[:],
                                 start=(d == 0), stop=(d == DT_T - 1))
            invb = sc.tile([128, TC], F32, tag="invb", bufs=1, name="invbf")
            nc.scalar.activation(invb[:], ssp[:], AF.Abs_reciprocal_sqrt,
                                 bias=RMS_EPS, scale=1.0 / D)
            for d in range(DT_T):
                hnf = sc.tile([128, TC], F32, tag="hnf", bufs=1, name="hnf")
                nc.vector.scalar_tensor_tensor(
                    hnf[:], h[d][:, tsl], nfw_c[d][:, 0:1], invb[:],
                    ALU.mult, ALU.mult)
                nc.vector.tensor_reduce(hmacc[d][:, c : c + 1], hnf[:],
                                        mybir.AxisListType.X, ALU.add)
        psf = psA.tile([NCLS, 1], F32, tag="mm", name="psfc")
        for d in range(DT_T):
            hms = sc.tile([128, 1], F32, tag="hms", name="hms")
            nc.vector.tensor_reduce(hms[:], hmacc[d][:], mybir.AxisListType.X,
                                    ALU.add)
            nc.tensor.matmul(psf[:], fcw_c[d][:], hms[:],
                             start=(d == 0), stop=(d == DT_T - 1))
        ot = sc.tile([NCLS, 1], F32, tag="ot", name="ot")
        nc.vector.scalar_tensor_tensor(ot[:], psf[:], 1.0 / L, fcb_t[:],
                                       ALU.mult, ALU.add)
        dma(out_dram[b].rearrange("(a one) -> a one", one=1), ot[:])

    stack.close()


def _conv_diag(cw):
    # conv_w [NL, E, 1, DCONV] -> per (l, e-tile, k) a 128x128 diagonal block:
    # out[l, et, p, k*128 + q] = cw[l, et*128+p, 0, k] if p == q else 0
    et = E // 128
    out = np.zeros((NL, et, 128, DCONV * 128), np.float32)
    for l in range(NL):
        for t in range(et):
            for k in range(DCONV):
                blk = np.diag(cw[l, t * 128 : (t + 1) * 128, 0, k])
                out[l, t, :, k * 128 : (k + 1) * 128] = blk
    return np.ascontiguousarray(out).astype(NPBF)


def _pad_xpw(xp):
    # [NL, 64, E] -> transposed+padded [NL, E, 96]: cols 0:48 = dt_raw+B,
    # cols 64:80 = C (PSUM partition bases must be multiples of 32)
    t = np.transpose(xp, (0, 2, 1))
    out = np.zeros((t.shape[0], t.shape[1], 96), t.dtype)
    out[:, :, 0:48] = t[:, :, 0:48]
    out[:, :, 64:80] = t[:, :, 48:64]
    return out


def _prep_inputs(inp):
    f32 = np.float32
    ca = np.ascontiguousarray

    def bf(a):
        return ca(np.asarray(a, f32)).astype(NPBF)

    bns = np.asarray(inp["bn_w"], f32) / np.sqrt(np.asarray(inp["bn_var"], f32) + BN_EPS)
    bnb = np.asarray(inp["bn_b"], f32) - np.asarray(inp["bn_mean"], f32) * bns
    base = {
        "dww": ca(np.asarray(inp["dw_w"], f32)[:, 0, :]),
        "pwT": ca(np.asarray(inp["pw_w"], f32)[:, :, 0].T),
        "bns": ca(bns),
        "bnb": ca(bnb),
        "nw": ca(np.asarray(inp["norm_w"], f32)),
        "inwT": bf(np.transpose(np.asarray(inp["in_proj_w"], f32), (0, 2, 1))),
        "cwdiag": _conv_diag(np.asarray(inp["conv_w"], f32)),
        "cb": ca(np.asarray(inp["conv_b"], f32)),
        "xpwT": bf(_pad_xpw(np.asarray(inp["xproj_w"], f32))),
        "dtpwT": bf(np.transpose(np.asarray(inp["dtproj_w"], f32), (0, 2, 1))),
        "dtpb": ca(np.asarray(inp["dtproj_b"], f32)),
        "A": ca(-np.exp(np.asarray(inp["A_log"], f32))),
        "Dp": ca(np.asarray(inp["D"], f32)),
        "owT": bf(np.transpose(np.asarray(inp["outproj_w"], f32), (0, 2, 1))),
        "nfw": ca(np.asarray(inp["normf_w"], f32)),
        "fcwT": ca(np.asarray(inp["fc_w"], f32).T),
        "fcb": ca(np.asarray(inp["fc_b"], f32)),
        "ones": np.ones((128, 128), NPBF),
        "ident": np.eye(128, dtype=np.float32).astype(NPBF),
        "sel": ca(np.repeat(np.eye(N, dtype=f32), 128, axis=1)).astype(NPBF),
    }
    x = np.asarray(inp["x"], f32)
    in_maps = []
    for i in range(NCORES):
        m = dict(base)
        m["xs"] = ca(x[i * BC : (i + 1) * BC])
        in_maps.append(m)
    return in_maps


_LAST_RESULTS = {}


def kernel(**inputs):
    nc = bacc.Bacc("TRN2", target_bir_lowering=False, debug=False,
                   num_devices=NCORES)
    dd, out_d = _declare(nc)
    with tile.TileContext(nc) as tcx:
        _body(tcx, dd, out_d)
    nc.compile()

    in_maps = _prep_inputs(inputs)
    trace = bool(os.environ.get("BASS_KERNEL_TRACE"))
    res = run_bass_kernel_spmd(nc, in_maps, list(range(NCORES)), trace=trace)
    _LAST_RESULTS["res"] = res
    if res.exec_time_ns is not None:
        print(f"HW exec time: {res.exec_time_ns} ns")
    out = np.concatenate(
        [np.asarray(res.results[i]["out"], np.float32) for i in range(NCORES)], axis=0)
    return out
